# revision 24
# baseline (speedup 1.0000x reference)
"""EpsBallPoints kernel for Trainium2 (8 NeuronCores, batch-parallel).

For each query s (B=8, S=2048) find the first NSAMPLE=64 point indices
(in increasing index order) among N=8192 points within RADIUS, padding
with the first valid index (or N if none).

Host prep per core (one batch element per core):
  - sort queries by x; each tile of 128 consecutive sorted queries only
    needs candidate points with x in [tile_min-R, tile_max+R] (a point
    within RADIUS of a query cannot differ by more than R in x).
  - per tile, gather those candidate points, RE-SORTED BY ORIGINAL ID,
    padded to W_CAND columns -> candidate order == id order.

Device algorithm per query tile:
  1. TensorE: augmented K=4 matmul  d_aug[m,j] = -2*s_m.c_j + ||c_j||^2
     valid  <=>  d_aug <= R^2 - ||s_m||^2  (per-partition threshold).
  2. ScalarE: mask = Relu(Sign(thr - d_aug)) in {0,1} (fp16).
  3. DVE: rank = inclusive cumsum of mask (tensor_tensor_scan, fp16 out
     is exact for ranks <= 2048 and clamps fine above 65).
  4. ScalarE: z = Relu(65 - rank): value (65-r) at the r-th valid point
     (r<=64), 0 once rank >= 65; duplicated values at invalid positions
     appear only AFTER the valid position carrying the same value.
  5. DVE: 8x max_index (FIND_INDEX_8) with constant needles 64..1 ->
     first occurrence of z==65-r is the column of the r-th valid point.
     Unfound -> 0xFFFF.
  6. Host: map window columns back to original ids, pad short rows,
     undo the query sort.
"""

import copy

import numpy as np

RADIUS = 0.2
NSAMPLE = 64
B, S, N = 8, 2048, 8192
P = 128              # queries per tile (partition dim)
NT = S // P          # 16 query tiles (4x4 spatial cells)
GX = 4               # query grid: GX x-strips x GY y-cells
GY = 4
NQ = 2048            # max PSUM chunk width (4 banks of fp32)
MM_N = 512           # matmul free width (one PSUM bank)

_CACHE = {}


def _chunks(w):
    """Split window width w (multiple of 512) into PSUM chunks <= NQ."""
    out = []
    o = 0
    while o < w:
        c = min(NQ, w - o)
        out.append((o, c))
        o += c
    return out


def _split_sync_waits(module, maxw=1):
    """Walrus in this toolchain rejects instructions carrying more than a
    couple of sem waits ("Too many sync wait commands"). Hoist excess waits
    onto single-wait NoOps placed immediately before, on the same engine."""
    from concourse import mybir

    for fn in module.functions:
        new_blocks = []
        for bb in fn.blocks:
            new_insts = []
            for inst in bb.instructions:
                si = inst.sync_info
                waits = list(si.on_wait) if si is not None else []
                if len(waits) > maxw:
                    k = 0
                    while len(waits) > maxw:
                        chunk, waits = waits[:maxw], waits[maxw:]
                        nop = mybir.InstNoOp(name=f"{inst.name}-w{k}")
                        k += 1
                        nop.engine = inst.engine
                        nop.sync_info = mybir.SyncInfo(on_wait=chunk, on_update=[])
                        new_insts.append(nop)
                    inst.sync_info = mybir.SyncInfo(
                        on_wait=waits, on_update=list(si.on_update)
                    )
                new_insts.append(inst)
            new_blocks.append(copy.replace(bb, instructions=new_insts))
        fn.blocks.clear()
        for b in new_blocks:
            fn.blocks.append(b)


def _build_program(widths, finalize=True):
    """widths: tuple of NT per-tile candidate-window widths (multiples of 512)."""
    key = ("nc", widths)
    if finalize and key in _CACHE:
        return _CACHE[key]
    import concourse.bass as bass
    from concourse import mybir
    from concourse.tile import TileContext

    f32 = mybir.dt.float32
    f16 = mybir.dt.float16
    u16 = mybir.dt.uint16
    Act = mybir.ActivationFunctionType
    Alu = mybir.AluOpType
    wmax = max(widths)
    offs = np.concatenate([[0], np.cumsum(widths)]).tolist()
    wtot = offs[-1]

    nc = bass.Bass()
    lhsT = nc.declare_dram_parameter("lhsT", [4, S], f32, isOutput=False)
    rhs = nc.declare_dram_parameter("rhs", [4, wtot], f32, isOutput=False)
    thr = nc.declare_dram_parameter("thr", [P, NT], f32, isOutput=False)
    ndl = nc.declare_dram_parameter("ndl", [P, NSAMPLE], f16, isOutput=False)
    out_idx = nc.declare_dram_parameter("out_idx", [S, NSAMPLE], u16, isOutput=True)
    out_cnt = nc.declare_dram_parameter("out_cnt", [S, 1], f32, isOutput=True)

    with TileContext(nc) as tc:
        with (
            tc.tile_pool(name="const", bufs=1) as cpool,
            tc.tile_pool(name="psum", bufs=2, space="PSUM") as ppool,
            tc.tile_pool(name="rhsp", bufs=3) as rpool,
            tc.tile_pool(name="work", bufs=3) as wpool,
            tc.tile_pool(name="outp", bufs=3) as opool,
        ):
            sb_lhsT = cpool.tile([4, S], f32)
            nc.sync.dma_start(out=sb_lhsT, in_=lhsT[:, :])
            sb_thr = cpool.tile([P, NT], f32)
            nc.sync.dma_start(out=sb_thr, in_=thr[:, :])
            sb_ndl = cpool.tile([P, NSAMPLE], f16)
            nc.sync.dma_start(out=sb_ndl, in_=ndl[:, :])

            for t in range(NT):
                w = widths[t]
                sb_rhs = rpool.tile([4, wmax], f32, tag="rhs")
                nc.sync.dma_start(out=sb_rhs[:, :w], in_=rhs[:, offs[t] : offs[t] + w])
                m01 = wpool.tile([P, wmax], f16, tag="m01")
                rank = wpool.tile([P, wmax], f16, tag="rank")
                cnt = opool.tile([P, 1], f32, tag="cnt")

                for q0, qw in _chunks(w):
                    ps = ppool.tile([P, NQ], f32, tag="ps")
                    for c0 in range(0, qw, MM_N):
                        cw = min(MM_N, qw - c0)
                        nc.tensor.matmul(
                            ps[:, c0 : c0 + cw],
                            sb_lhsT[:, t * P : (t + 1) * P],
                            sb_rhs[:, q0 + c0 : q0 + c0 + cw],
                            start=True,
                            stop=True,
                        )
                    # sign in {-1,0,1}: Sign(thr - d_aug)
                    nc.scalar.activation(
                        out=m01[:, q0 : q0 + qw],
                        in_=ps[:, :qw],
                        func=Act.Sign,
                        bias=sb_thr[:, t : t + 1],
                        scale=-1.0,
                    )
                # mask {0,1}; accumulate per-row count on the fly
                nc.scalar.activation(
                    out=m01[:, :w], in_=m01[:, :w], func=Act.Relu, accum_out=cnt
                )
                # rank = inclusive cumsum(mask) over the whole window
                nc.vector.tensor_tensor_scan(
                    out=rank[:, :w],
                    data0=m01[:, :w],
                    data1=m01[:, :w],
                    initial=0.0,
                    op0=Alu.add,
                    op1=Alu.bypass,
                )
                # find positions of ranks 1..64 directly (first occurrence of
                # rank==r is the r-th valid column; fp16 rank is exact <= 2048
                # and larger ranks can never round into the needle range)
                idx = opool.tile([P, NSAMPLE], u16, tag="idx")
                for j in range(NSAMPLE // 8):
                    nc.vector.max_index(
                        out=idx[:, 8 * j : 8 * (j + 1)],
                        in_max=sb_ndl[:, 8 * j : 8 * (j + 1)],
                        in_values=rank[:, :w],
                    )
                nc.sync.dma_start(out=out_idx[t * P : (t + 1) * P, :], in_=idx)
                nc.sync.dma_start(out=out_cnt[t * P : (t + 1) * P, :], in_=cnt)

    if not finalize:
        return nc
    nc.finalize()
    _split_sync_waits(nc.m)
    _CACHE[key] = nc
    return nc


def _prep_core_phase1(samples_b, coord_b):
    """2D (x,y)-cell query ordering + per-tile candidate id lists."""
    sx = np.asarray(samples_b, dtype=np.float32)
    cx = np.asarray(coord_b, dtype=np.float32)

    # sort queries into GX x-strips, each y-sorted into GY cells of P queries
    xorder = np.argsort(sx[:, 0], kind="stable")
    qorder = np.empty(S, np.int64)
    strip = S // GX
    for g in range(GX):
        idx = xorder[g * strip : (g + 1) * strip]
        yo = idx[np.argsort(sx[idx, 1], kind="stable")]
        qorder[g * strip : (g + 1) * strip] = yo
    qs = sx[qorder]

    cands = []
    for t in range(NT):
        q = qs[t * P : (t + 1) * P]
        xlo, xhi = q[:, 0].min(), q[:, 0].max()
        ylo, yhi = q[:, 1].min(), q[:, 1].max()
        # 2D distance from the cell's query bounding rect must be <= RADIUS
        dx = np.maximum(0.0, np.maximum(xlo - cx[:, 0], cx[:, 0] - xhi))
        dy = np.maximum(0.0, np.maximum(ylo - cx[:, 1], cx[:, 1] - yhi))
        m = dx * dx + dy * dy <= RADIUS * RADIUS
        cands.append(np.flatnonzero(m))  # ascending original ids
    return qs, qorder, cands, cx


def _prep_core_phase2(qs, cands, cx, widths):
    offs = np.concatenate([[0], np.cumsum(widths)])
    wtot = int(offs[-1])
    wmax = max(widths)
    lhsT = np.empty((4, S), np.float32)
    lhsT[0:3] = qs.T
    lhsT[3] = 1.0
    ss = (qs * qs).sum(axis=1)
    thr = np.ascontiguousarray(
        (RADIUS * RADIUS - ss).reshape(NT, P).T, dtype=np.float32
    )
    ndl = np.broadcast_to(
        np.arange(1, NSAMPLE + 1, dtype=np.float16)[None, :], (P, NSAMPLE)
    ).copy()
    rhs = np.zeros((4, wtot), np.float32)
    rhs[3, :] = 1e9  # padding: huge ||c||^2 -> never within radius
    lut = np.full((NT, wmax), N, np.int32)
    for t in range(NT):
        cand = cands[t]
        w = len(cand)
        cc = cx[cand]
        o = int(offs[t])
        rhs[0:3, o : o + w] = -2.0 * cc.T
        rhs[3, o : o + w] = (cc * cc).sum(axis=1)
        lut[t, :w] = cand
    return {"lhsT": lhsT, "rhs": rhs, "thr": thr, "ndl": ndl}, lut


def _postprocess_core(idx_u16, cnt_f32, qorder, lut):
    wmax = lut.shape[1]
    idx = idx_u16.astype(np.int64)  # [S, 64] window columns; 65535 unfound
    cnt = cnt_f32.reshape(S).astype(np.int32)
    kk = np.arange(NSAMPLE, dtype=np.int32)[None, :]
    valid = kk < np.minimum(cnt, NSAMPLE)[:, None]
    tiles = np.repeat(np.arange(NT), P)  # sorted-query row -> tile
    mapped = lut[tiles[:, None], np.minimum(idx, wmax - 1)]  # [S, 64]
    first = np.where(cnt[:, None] >= 1, mapped[:, :1], N)
    out_sorted = np.where(valid, mapped, first).astype(np.int32)
    out = np.empty_like(out_sorted)
    out[qorder] = out_sorted
    return out


def kernel(samples: np.ndarray, coord: np.ndarray, _want_trace: bool = False):
    from concourse.bass_utils import run_bass_kernel_spmd

    samples = np.asarray(samples, dtype=np.float32)
    coord = np.asarray(coord, dtype=np.float32)
    core_ids = list(range(B))
    phase1 = [_prep_core_phase1(samples[b], coord[b]) for b in range(B)]
    widths = tuple(
        max(256, int(-(-max(len(phase1[b][2][t]) for b in range(B)) // 256) * 256))
        for t in range(NT)
    )
    nc = _build_program(widths)
    in_maps = []
    luts = []
    for b in range(B):
        qs, qorder, cands, cx = phase1[b]
        im, lut = _prep_core_phase2(qs, cands, cx, widths)
        in_maps.append(im)
        luts.append(lut)
    res = run_bass_kernel_spmd(nc, in_maps, core_ids, trace=_want_trace)

    out = np.empty((B, S, NSAMPLE), np.int32)
    for b in range(B):
        out[b] = _postprocess_core(
            res.results[b]["out_idx"],
            res.results[b]["out_cnt"],
            phase1[b][1],
            luts[b],
        )
    if _want_trace:
        return out, res
    return out
